# revision 44
# baseline (speedup 1.0000x reference)
"""Multi-head self-attention (B=4, S=2048, D=1024, 16 heads x 64) on 8 TRN2
NeuronCores via Bass/Tile.

Sharding: tensor-parallel over heads. Each core owns 2 heads (128 of the 1024
Q/K/V output features, column-parallel) and the matching 128 rows of Wo
(row-parallel). Every core computes a full-shape partial output in bf16; the
host sums the 8 partials (the row-parallel all-reduce) and adds bo once.

Per-core dataflow (matmul operands bf16, score PSUM bf16, PV/psum acc fp32):
  xT[b]   : [D, S] features-on-partitions (host pre-transposed)
  qT/kT/vT: [128, S]  = (x @ W)^T per core, via lhsT=W k-tiles, rhs=xT
  v_aug   : PE-transpose of vT -> v natural [S,64] per head + em column,
            where em[j] = exp(mask[j]) folds the additive attention mask
            into the value/denominator stream (exp(s+m) = exp(s)*em)
  scoresT : [j, q] per 2-jt group in ONE [128, 2048] bf16 PSUM tile; the two
            heads occupy PE row-groups (0-63 / 64-127), 4 matmuls per group
  exp     : one ACT Exp op per 2-jt group [128, 2048] (scale=1/8, no bias)
  PV      : lhsT=[v_h*em | em] [128 j, 65], rhs=exp half, accumulated over
            j-tiles -> rows 0-63 ctx^T (mask-weighted), row 64 = denominator
  norm    : reciprocal_approx_fast of denom row + GPSIMD partition
            broadcast, multiply -> ctxT [128, S] bf16
  out     : lhsT=ctxT tile [128,128], rhs=Wo_c [128,512] chunks; DVE adds
            bo/8 while converting PSUM->SBUF bf16; DMA partial to DRAM

The emission is software-pipelined: batch b's attention steps are interleaved
with batch b+1's QKV/V-transpose units and batch b-1's output-projection
units, so the (in-order) PE always has independent matmul work while the
ACT-bound softmax stream runs, keeping the PE HAM clock-gate warm.
"""

import numpy as np
import ml_dtypes

import concourse.bass as bass
import concourse.mybir as mybir
import concourse.tile as tile
from concourse import bacc, bass_utils
from concourse.masks import make_identity

F32 = mybir.dt.float32
BF16 = mybir.dt.bfloat16
AF = mybir.ActivationFunctionType
BF = ml_dtypes.bfloat16
ts = bass.ts

B, S, D = 4, 2048, 1024
NH, HD = 16, 64
NCORES = 8
OF = D // NCORES            # 128 out-features per core (2 heads)
NKT = D // 128              # 8 contraction tiles
NJT = S // 128              # 16 key tiles per batch
NICH = S // 512             # 4 query chunks per batch
NTT = S // 128              # 16 token tiles per batch
NG = NJT // 2               # 8 two-jt groups per query chunk


def build_program():
    nc = bacc.Bacc("TRN2", target_bir_lowering=False, debug=False,
                   num_devices=NCORES)
    xT_d = nc.dram_tensor("xT", [B, D, S], BF16, kind="ExternalInput")
    wq_d = nc.dram_tensor("wq", [128, NKT, OF], BF16, kind="ExternalInput")
    wk_d = nc.dram_tensor("wk", [128, NKT, OF], BF16, kind="ExternalInput")
    wv_d = nc.dram_tensor("wv", [128, NKT, OF], BF16, kind="ExternalInput")
    bq_d = nc.dram_tensor("bq", [OF, 1], F32, kind="ExternalInput")
    bk_d = nc.dram_tensor("bk", [OF, 1], F32, kind="ExternalInput")
    bv_d = nc.dram_tensor("bv", [OF, 1], F32, kind="ExternalInput")
    wo_d = nc.dram_tensor("wo", [OF, D], BF16, kind="ExternalInput")
    emcol_d = nc.dram_tensor("emcol", [128, B * NJT, 1], F32,
                             kind="ExternalInput")
    emrow_d = nc.dram_tensor("emrow", [1, B, S], F32, kind="ExternalInput")
    out_d = nc.dram_tensor("out", [B * S, D], BF16, kind="ExternalOutput")

    with tile.TileContext(nc) as tc:
        with (
            tc.tile_pool(name="consts", bufs=1) as consts,
            tc.tile_pool(name="xin", bufs=3) as xin,
            tc.tile_pool(name="vstg", bufs=4) as vstg,
            tc.tile_pool(name="qkv", bufs=2) as qkv,
            tc.tile_pool(name="attn", bufs=4) as attn,
            tc.tile_pool(name="ctxp", bufs=2) as ctxp,
            tc.tile_pool(name="outp", bufs=6) as outp,
            tc.tile_pool(name="psum", bufs=2, space="PSUM") as psum,
        ):
            # ---------------- constants ----------------
            # DMA issue order tuned for the first matmul's dependencies:
            # em (tiny, feeds the va ones column and the V em-scaling),
            # then wq; the first x chunk is issued by pull(g_qkv[0], 1)
            # below, before the remaining weights which are needed later.
            ident = consts.tile([128, 128], BF16)
            make_identity(nc, ident)
            em_sb = consts.tile([128, B * NJT, 1], F32)
            nc.sync.dma_start(em_sb, emcol_d[:, :, :])
            emrow_sb = consts.tile([1, B, S], F32)
            nc.sync.dma_start(emrow_sb, emrow_d[:, :, :])
            w_sb = {}
            for nm, d in (("q", wq_d), ("k", wk_d), ("v", wv_d)):
                w_sb[nm] = consts.tile([128, NKT, OF], BF16, name=f"w{nm}_sb")
            b_sb = {}
            for nm, d in (("q", bq_d), ("k", bk_d), ("v", bv_d)):
                b_sb[nm] = consts.tile([OF, 1], F32, name=f"b{nm}_sb")
            nc.sync.dma_start(w_sb["q"], wq_d[:, :, :])
            wo_sb = consts.tile([OF, D], BF16)

            def late_const_dmas():
                nc.sync.dma_start(w_sb["k"], wk_d[:, :, :])
                nc.sync.dma_start(w_sb["v"], wv_d[:, :, :])
                nc.sync.dma_start(b_sb["q"], bq_d[:, :])
                nc.sync.dma_start(b_sb["k"], bk_d[:, :])
                nc.sync.dma_start(b_sb["v"], bv_d[:, :])
                nc.sync.dma_start(wo_sb, wo_d[:, :])

            state = [dict() for _ in range(B)]

            def qkv_units(b):
                """QKV projections for batch b. V is em-scaled in the bias
                op and transposed into v_aug via DMA XBAR (no PE/DVE work).
                14 yields per chunk, 56 per batch."""
                st = state[b]
                pT = {nm: qkv.tile([OF, S], BF16, name=f"{nm}T")
                      for nm in ("q", "k", "v")}
                st["pT"] = pT
                va = [qkv.tile([128, NJT, 65], BF16, name=f"v_aug{h}")
                      for h in range(2)]
                st["va"] = va
                for h in range(2):
                    nc.vector.tensor_copy(
                        va[h][:, :, 64:65],
                        em_sb[:, b * NJT:(b + 1) * NJT, :])
                rep = qkv.tile([128, S], F32, name="em_rep")
                nc.gpsimd.partition_broadcast(rep, emrow_sb[0:1, b, :])
                for nch in range(NICH):
                    xt = xin.tile([128, NKT, 512], BF16, name="xt")
                    xr = xT_d[b].rearrange("(k p) t -> p k t", p=128)
                    for kt in range(0, NKT, 2):
                        nc.sync.dma_start(
                            xt[:, kt:kt + 2, :], xr[:, kt:kt + 2, ts(nch, 512)])
                    yield
                    for nm in ("q", "k", "v"):
                        ps = psum.tile([128, 512], F32, tag="mm",
                                       name="ps_qkv")
                        for kt in range(NKT):
                            nc.tensor.matmul(
                                ps, lhsT=w_sb[nm][:, kt, :],
                                rhs=xt[:, kt, :],
                                start=(kt == 0), stop=(kt == NKT - 1),
                            )
                            if kt % 2 == 1:
                                yield
                        if nm == "v":
                            nc.vector.scalar_tensor_tensor(
                                pT[nm][:, ts(nch, 512)], ps, b_sb[nm],
                                rep[:, ts(nch, 512)],
                                op0=mybir.AluOpType.add,
                                op1=mybir.AluOpType.mult)
                        else:
                            nc.vector.tensor_scalar_add(
                                pT[nm][:, ts(nch, 512)], ps, b_sb[nm])
                    for jt in range(4 * nch, 4 * nch + 4):
                        pvt = psum.tile([128, 128], BF16, tag="mm",
                                        name="pvt")
                        nc.tensor.transpose(
                            pvt, pT["v"][:, ts(jt, 128)], ident)
                        for h in range(2):
                            nc.vector.tensor_copy(
                                va[h][:, jt, 0:64],
                                pvt[:, h * 64:(h + 1) * 64])
                        yield

            def attn_units(b):
                """Attention + normalize for batch b. 18 yields per query
                chunk, 72 per batch."""
                st = state[b]
                qT, kT = st["pT"]["q"], st["pT"]["k"]
                va = st["va"]
                ctxT = ctxp.tile([128, S], BF16, name="ctxT")
                st["ctxT"] = ctxT

                for ich in range(NICH):
                    isl = ts(ich, 512)
                    pc = [psum.tile([128, 512], F32, tag="pc", name=f"pc{h}")
                          for h in range(2)]
                    pend_pv = []

                    def emit_pv(jt, et):
                        for h in range(2):
                            nc.tensor.matmul(
                                pc[h][0:65, :], lhsT=va[h][:, jt, :],
                                rhs=et[:, ts(h, 512)],
                                start=(jt == 0), stop=(jt == NJT - 1),
                            )

                    for jp in range(0, NJT, 2):
                        # two j-tiles of scores back-to-back: their four
                        # row-group-alternating matmuls keep LDWEIGHTS
                        # pull-ahead unblocked (no K=128 matmul between)
                        scs = []
                        for jt in (jp, jp + 1):
                            sc = psum.tile([128, 1024], F32, tag="sc",
                                           name="sc")
                            for h in range(2):
                                hs = slice(h * 64, (h + 1) * 64)
                                nc.tensor.matmul(
                                    sc[:, ts(h, 512)],
                                    lhsT=kT[hs, ts(jt, 128)],
                                    rhs=qT[hs, isl],
                                    start=True, stop=True,
                                )
                            scs.append(sc)
                        for idx, jt in enumerate((jp, jp + 1)):
                            et = attn.tile([128, 1024], BF16, name="et",
                                           bufs=8)
                            nc.scalar.activation(
                                et, scs[idx], AF.Exp, scale=0.125)
                            pend_pv.append((jt, et))
                            if len(pend_pv) > 4:
                                emit_pv(*pend_pv.pop(0))
                            yield
                    while pend_pv:
                        emit_pv(*pend_pv.pop(0))
                    # normalize: interleave the two heads' chains so the
                    # DVE rec ops pipeline with the gpsimd broadcasts
                    recs, reps = [], []
                    for h in range(2):
                        den = attn.tile([1, 512], F32, name=f"den{h}")
                        nc.vector.tensor_copy(den, pc[h][64:65, :])
                        rec = attn.tile([1, 512], F32, name=f"rec{h}")
                        nc.vector.reciprocal_approx_fast(rec, den)
                        recs.append(rec)
                    for h in range(2):
                        rep = attn.tile([64, 512], F32, name=f"rep{h}")
                        nc.gpsimd.partition_broadcast(rep, recs[h])
                        reps.append(rep)
                    yield
                    for h in range(2):
                        nc.vector.tensor_mul(
                            ctxT[h * 64:(h + 1) * 64, isl],
                            pc[h][0:64, :], reps[h])
                    yield

            def outproj_units(b):
                """Output projection for batch b (host adds bo). The last
                batch's late staging copies go through ACT, which is idle
                by then, so DVE stays free for the final normalizes.
                32 yields."""
                ctxT = state[b]["ctxT"]
                u = 0
                for tt in range(NTT):
                    for oc in range(2):
                        po = psum.tile([128, 512], F32, tag="mm", name="po")
                        nc.tensor.matmul(
                            po, lhsT=ctxT[:, ts(tt, 128)],
                            rhs=wo_sb[:, ts(oc, 512)],
                            start=True, stop=True,
                        )
                        osb = outp.tile([128, 512], BF16, name="osb")
                        if b == B - 1 and u >= 24 and u % 2 == 0:
                            nc.scalar.activation(osb, po, AF.Copy)
                        else:
                            nc.vector.tensor_copy(osb, po)
                        nc.sync.dma_start(
                            out_d[b * S + tt * 128: b * S + (tt + 1) * 128,
                                  ts(oc, 512)],
                            osb)
                        u += 1
                        yield

            def drain(*weighted):
                """weighted: (gen, stride[, delay]) — advance gen every
                `stride` cycles after `delay` cycles. Run until exhausted."""
                live = []
                for w in weighted:
                    g, s, d = (w + (0,)) if len(w) == 2 else w
                    if g is not None:
                        live.append((g, s, d))
                cyc = 0
                while live:
                    nxt = []
                    for g, s, d in live:
                        if cyc >= d and (cyc - d) % s == 0:
                            try:
                                next(g)
                            except StopIteration:
                                continue
                        nxt.append((g, s, d))
                    live = nxt
                    cyc += 1

            def pull(g, n):
                for _ in range(n):
                    try:
                        next(g)
                    except StopIteration:
                        return False
                return True

            g_attn = [attn_units(b) for b in range(B)]
            g_qkv = [qkv_units(b) for b in range(B)]
            g_out = [outproj_units(b) for b in range(B)]

            # prologue: batch 0 QKV chunk 0 first (its x DMA before wo/bo),
            # then pace attention(0) in at 1:3 while QKV(0) streams and
            # QKV(1) trickles in to fill attention(0)'s ACT-bound gaps
            pull(g_qkv[0], 1)
            late_const_dmas()
            pull(g_qkv[0], 11)
            drain((g_qkv[0], 1), (g_attn[0], 4), (g_qkv[1], 3, 52))
            for b in range(1, B):
                drain(
                    (g_qkv[b + 1] if b + 1 < B else None, 1),
                    (g_attn[b], 1),
                    (g_out[b - 1], 2, 10),
                    (g_out[b] if b == B - 1 else None, 2, 24),
                )
            drain((g_out[B - 1], 1))
    nc.finalize()
    return nc


def make_in_maps(x, attention_mask, Wq, bq, Wk, bk, Wv, bv, Wo, bo):
    x = np.asarray(x, dtype=np.float32)
    attention_mask = np.asarray(attention_mask, dtype=np.float32)
    Wq, Wk, Wv, Wo = (np.asarray(a, dtype=np.float32) for a in (Wq, Wk, Wv, Wo))
    bq, bk, bv, bo = (np.asarray(a, dtype=np.float32) for a in (bq, bk, bv, bo))

    xT = np.ascontiguousarray(x.transpose(0, 2, 1)).astype(BF)  # [B, D, S]
    # exp(mask[b,0,0,j]): [128 partitions, B*NJT, 1] column per (batch,
    # j-tile) for the va ones column, and [1, B, S] row layout for the
    # V em-scaling (folds the additive mask into the value/denom stream)
    m = attention_mask.reshape(B, S).reshape(B, NJT, 128)
    emcol = np.ascontiguousarray(
        np.exp(m.transpose(2, 0, 1).reshape(128, B * NJT, 1)))
    emrow = np.ascontiguousarray(
        np.exp(attention_mask.reshape(1, B, S)))

    def w3(W, cs):
        # [D, OF] -> [128, NKT, OF]: partition p holds rows {kt*128 + p}
        return np.ascontiguousarray(
            W[:, cs].reshape(NKT, 128, OF).transpose(1, 0, 2)).astype(BF)

    in_maps = []
    for c in range(NCORES):
        cs = slice(c * OF, (c + 1) * OF)
        in_maps.append({
            "xT": xT,
            "wq": w3(Wq, cs),
            "wk": w3(Wk, cs),
            "wv": w3(Wv, cs),
            "bq": np.ascontiguousarray(bq[cs]).reshape(OF, 1),
            "bk": np.ascontiguousarray(bk[cs]).reshape(OF, 1),
            "bv": np.ascontiguousarray(bv[cs]).reshape(OF, 1),
            "wo": np.ascontiguousarray(Wo[cs, :]).astype(BF),
            "emcol": emcol,
            "emrow": emrow,
        })
    return in_maps


def combine_outputs(results, bo):
    acc = np.zeros((B * S, D), dtype=np.float64)
    for r in results:
        acc += r["out"].astype(np.float64)
    acc += np.asarray(bo, dtype=np.float64)
    return acc.reshape(B, S, D).astype(np.float32)


_NC_CACHE = []


def _get_program():
    if not _NC_CACHE:
        _NC_CACHE.append(build_program())
    return _NC_CACHE[0]


def kernel(**inputs):
    nc = _get_program()
    in_maps = make_in_maps(**inputs)
    res = bass_utils.run_bass_kernel_spmd(
        nc, in_maps, core_ids=list(range(NCORES)))
    return combine_outputs(res.results, inputs["bo"])


# revision 45
# speedup vs baseline: 1.1896x; 1.1896x over previous
"""Multi-head self-attention (B=4, S=2048, D=1024, 16 heads x 64) on 8 TRN2
NeuronCores via Bass/Tile.

Sharding: tensor-parallel over heads. Each core owns 2 heads (128 of the 1024
Q/K/V output features, column-parallel) and the matching 128 rows of Wo
(row-parallel). Every core computes a full-shape partial output in bf16; the
host sums the 8 partials (the row-parallel all-reduce) and adds bo once.

Per-core dataflow (matmul operands bf16, score PSUM bf16, PV/psum acc fp32):
  xT[b]   : [D, S] features-on-partitions (host pre-transposed)
  qT/kT/vT: [128, S]  = (x @ W)^T per core, via lhsT=W k-tiles, rhs=xT
  v_aug   : PE-transpose of vT -> v natural [S,64] per head + em column,
            where em[j] = exp(mask[j]) folds the additive attention mask
            into the value/denominator stream (exp(s+m) = exp(s)*em)
  scoresT : [j, q] per 2-jt group in ONE [128, 2048] bf16 PSUM tile; the two
            heads occupy PE row-groups (0-63 / 64-127), 4 matmuls per group
  exp     : one ACT Exp op per 2-jt group [128, 2048] (scale=1/8, no bias)
  PV      : lhsT=[v_h*em | em] [128 j, 65], rhs=exp half, accumulated over
            j-tiles -> rows 0-63 ctx^T (mask-weighted), row 64 = denominator
  norm    : reciprocal_approx_fast of denom row + GPSIMD partition
            broadcast, multiply -> ctxT [128, S] bf16
  out     : lhsT=ctxT tile [128,128], rhs=Wo_c [128,512] chunks; DVE adds
            bo/8 while converting PSUM->SBUF bf16; DMA partial to DRAM

The emission is software-pipelined: batch b's attention steps are interleaved
with batch b+1's QKV/V-transpose units and batch b-1's output-projection
units, so the (in-order) PE always has independent matmul work while the
ACT-bound softmax stream runs, keeping the PE HAM clock-gate warm.
"""

import numpy as np
import ml_dtypes

import concourse.bass as bass
import concourse.mybir as mybir
import concourse.tile as tile
from concourse import bacc, bass_utils
from concourse.masks import make_identity

F32 = mybir.dt.float32
BF16 = mybir.dt.bfloat16
AF = mybir.ActivationFunctionType
BF = ml_dtypes.bfloat16
ts = bass.ts

B, S, D = 4, 2048, 1024
NH, HD = 16, 64
NCORES = 8
OF = D // NCORES            # 128 out-features per core (2 heads)
NKT = D // 128              # 8 contraction tiles
NJT = S // 128              # 16 key tiles per batch
NICH = S // 512             # 4 query chunks per batch
NTT = S // 128              # 16 token tiles per batch
NG = NJT // 2               # 8 two-jt groups per query chunk


def build_program():
    nc = bacc.Bacc("TRN2", target_bir_lowering=False, debug=False,
                   num_devices=NCORES)
    xT_d = nc.dram_tensor("xT", [B, D, S], BF16, kind="ExternalInput")
    wq_d = nc.dram_tensor("wq", [128, NKT, OF], BF16, kind="ExternalInput")
    wk_d = nc.dram_tensor("wk", [128, NKT, OF], BF16, kind="ExternalInput")
    wv_d = nc.dram_tensor("wv", [128, NKT, OF], BF16, kind="ExternalInput")
    bq_d = nc.dram_tensor("bq", [OF, 1], F32, kind="ExternalInput")
    bk_d = nc.dram_tensor("bk", [OF, 1], F32, kind="ExternalInput")
    bv_d = nc.dram_tensor("bv", [OF, 1], F32, kind="ExternalInput")
    wo_d = nc.dram_tensor("wo", [OF, D], BF16, kind="ExternalInput")
    emcol_d = nc.dram_tensor("emcol", [128, B * NJT, 1], F32,
                             kind="ExternalInput")
    emrow_d = nc.dram_tensor("emrow", [1, B, S], F32, kind="ExternalInput")
    out_d = nc.dram_tensor("out", [B * S, D], BF16, kind="ExternalOutput")

    with tile.TileContext(nc) as tc:
        with (
            tc.tile_pool(name="consts", bufs=1) as consts,
            tc.tile_pool(name="xin", bufs=3) as xin,
            tc.tile_pool(name="vstg", bufs=4) as vstg,
            tc.tile_pool(name="qkv", bufs=2) as qkv,
            tc.tile_pool(name="attn", bufs=4) as attn,
            tc.tile_pool(name="ctxp", bufs=2) as ctxp,
            tc.tile_pool(name="outp", bufs=6) as outp,
            tc.tile_pool(name="psum", bufs=2, space="PSUM") as psum,
        ):
            # ---------------- constants ----------------
            # DMA issue order tuned for the first matmul's dependencies:
            # em (tiny, feeds the va ones column and the V em-scaling),
            # then wq; the first x chunk is issued by pull(g_qkv[0], 1)
            # below, before the remaining weights which are needed later.
            ident = consts.tile([128, 128], BF16)
            make_identity(nc, ident)
            em_sb = consts.tile([128, B * NJT, 1], F32)
            nc.sync.dma_start(em_sb, emcol_d[:, :, :])
            emrow_sb = consts.tile([1, B, S], F32)
            nc.sync.dma_start(emrow_sb, emrow_d[:, :, :])
            w_sb = {}
            for nm, d in (("q", wq_d), ("k", wk_d), ("v", wv_d)):
                w_sb[nm] = consts.tile([128, NKT, OF], BF16, name=f"w{nm}_sb")
            b_sb = {}
            for nm, d in (("q", bq_d), ("k", bk_d), ("v", bv_d)):
                b_sb[nm] = consts.tile([OF, 1], F32, name=f"b{nm}_sb")
            nc.sync.dma_start(w_sb["q"], wq_d[:, :, :])
            wo_sb = consts.tile([OF, D], BF16)

            def late_const_dmas():
                nc.sync.dma_start(w_sb["k"], wk_d[:, :, :])
                nc.sync.dma_start(w_sb["v"], wv_d[:, :, :])
                nc.sync.dma_start(b_sb["q"], bq_d[:, :])
                nc.sync.dma_start(b_sb["k"], bk_d[:, :])
                nc.sync.dma_start(b_sb["v"], bv_d[:, :])
                nc.sync.dma_start(wo_sb, wo_d[:, :])

            state = [dict() for _ in range(B)]

            def qkv_units(b):
                """QKV projections for batch b. V is em-scaled in the bias
                op and transposed into v_aug via DMA XBAR (no PE/DVE work).
                14 yields per chunk, 56 per batch."""
                st = state[b]
                pT = {nm: qkv.tile([OF, S], BF16, name=f"{nm}T")
                      for nm in ("q", "k", "v")}
                st["pT"] = pT
                va = [qkv.tile([128, NJT, 65], BF16, name=f"v_aug{h}")
                      for h in range(2)]
                st["va"] = va
                for h in range(2):
                    nc.vector.tensor_copy(
                        va[h][:, :, 64:65],
                        em_sb[:, b * NJT:(b + 1) * NJT, :])
                rep = qkv.tile([128, S], F32, name="em_rep")
                nc.gpsimd.partition_broadcast(rep, emrow_sb[0:1, b, :])
                for nch in range(NICH):
                    xt = xin.tile([128, NKT, 512], BF16, name="xt")
                    xr = xT_d[b].rearrange("(k p) t -> p k t", p=128)
                    for kt in range(0, NKT, 2):
                        nc.sync.dma_start(
                            xt[:, kt:kt + 2, :], xr[:, kt:kt + 2, ts(nch, 512)])
                    yield
                    for nm in ("q", "k", "v"):
                        ps = psum.tile([128, 512], F32, tag="mm",
                                       name="ps_qkv")
                        for kt in range(NKT):
                            nc.tensor.matmul(
                                ps, lhsT=w_sb[nm][:, kt, :],
                                rhs=xt[:, kt, :],
                                start=(kt == 0), stop=(kt == NKT - 1),
                            )
                            if kt % 2 == 1:
                                yield
                        if nm == "v":
                            nc.vector.scalar_tensor_tensor(
                                pT[nm][:, ts(nch, 512)], ps, b_sb[nm],
                                rep[:, ts(nch, 512)],
                                op0=mybir.AluOpType.add,
                                op1=mybir.AluOpType.mult)
                        else:
                            nc.vector.tensor_scalar_add(
                                pT[nm][:, ts(nch, 512)], ps, b_sb[nm])
                    for jt in range(4 * nch, 4 * nch + 4):
                        pvt = psum.tile([128, 128], BF16, tag="mm",
                                        name="pvt")
                        nc.tensor.transpose(
                            pvt, pT["v"][:, ts(jt, 128)], ident)
                        for h in range(2):
                            nc.vector.tensor_copy(
                                va[h][:, jt, 0:64],
                                pvt[:, h * 64:(h + 1) * 64])
                        yield

            def attn_units(b):
                """Attention + normalize for batch b. 18 yields per query
                chunk, 72 per batch."""
                st = state[b]
                qT, kT = st["pT"]["q"], st["pT"]["k"]
                va = st["va"]
                ctxT = ctxp.tile([128, S], BF16, name="ctxT")
                st["ctxT"] = ctxT

                for ich in range(NICH):
                    isl = ts(ich, 512)
                    pc = [psum.tile([128, 512], F32, tag="pc", name=f"pc{h}")
                          for h in range(2)]
                    pend_pv = []

                    def emit_pv(jt, et):
                        for h in range(2):
                            nc.tensor.matmul(
                                pc[h][0:65, :], lhsT=va[h][:, jt, :],
                                rhs=et[:, ts(h, 512)],
                                start=(jt == 0), stop=(jt == NJT - 1),
                            )

                    for jp in range(0, NJT, 2):
                        # two j-tiles of scores back-to-back: their four
                        # row-group-alternating matmuls keep LDWEIGHTS
                        # pull-ahead unblocked (no K=128 matmul between)
                        scs = []
                        for jt in (jp, jp + 1):
                            sc = psum.tile([128, 1024], F32, tag="sc",
                                           name="sc")
                            for h in range(2):
                                hs = slice(h * 64, (h + 1) * 64)
                                nc.tensor.matmul(
                                    sc[:, ts(h, 512)],
                                    lhsT=kT[hs, ts(jt, 128)],
                                    rhs=qT[hs, isl],
                                    start=True, stop=True,
                                )
                            scs.append(sc)
                        for idx, jt in enumerate((jp, jp + 1)):
                            et = attn.tile([128, 1024], BF16, name="et",
                                           bufs=8)
                            nc.scalar.activation(
                                et, scs[idx], AF.Exp, scale=0.125)
                            pend_pv.append((jt, et))
                            if len(pend_pv) > 4:
                                emit_pv(*pend_pv.pop(0))
                            yield
                    while pend_pv:
                        emit_pv(*pend_pv.pop(0))
                    # normalize: interleave the two heads' chains so the
                    # DVE rec ops pipeline with the gpsimd broadcasts
                    recs, reps = [], []
                    for h in range(2):
                        den = attn.tile([1, 512], F32, name=f"den{h}")
                        nc.vector.tensor_copy(den, pc[h][64:65, :])
                        rec = attn.tile([1, 512], F32, name=f"rec{h}")
                        nc.vector.reciprocal_approx_fast(rec, den)
                        recs.append(rec)
                    for h in range(2):
                        rep = attn.tile([64, 512], F32, name=f"rep{h}")
                        nc.gpsimd.partition_broadcast(rep, recs[h])
                        reps.append(rep)
                    yield
                    for h in range(2):
                        nc.vector.tensor_mul(
                            ctxT[h * 64:(h + 1) * 64, isl],
                            pc[h][0:64, :], reps[h])
                    yield

            def outproj_units(b):
                """Output projection for batch b (host adds bo). The last
                batch's late staging copies go through ACT, which is idle
                by then, so DVE stays free for the final normalizes.
                32 yields."""
                ctxT = state[b]["ctxT"]
                u = 0
                for tt in range(NTT):
                    for oc in range(2):
                        po = psum.tile([128, 512], F32, tag="mm", name="po")
                        nc.tensor.matmul(
                            po, lhsT=ctxT[:, ts(tt, 128)],
                            rhs=wo_sb[:, ts(oc, 512)],
                            start=True, stop=True,
                        )
                        osb = outp.tile([128, 512], BF16, name="osb")
                        if b == B - 1 and u >= 24 and u % 2 == 0:
                            nc.scalar.activation(osb, po, AF.Copy)
                        else:
                            nc.vector.tensor_copy(osb, po)
                        nc.sync.dma_start(
                            out_d[b * S + tt * 128: b * S + (tt + 1) * 128,
                                  ts(oc, 512)],
                            osb)
                        u += 1
                        yield

            def drain(*weighted):
                """weighted: (gen, stride[, delay]) — advance gen every
                `stride` cycles after `delay` cycles. Run until exhausted."""
                live = []
                for w in weighted:
                    g, s, d = (w + (0,)) if len(w) == 2 else w
                    if g is not None:
                        live.append((g, s, d))
                cyc = 0
                while live:
                    nxt = []
                    for g, s, d in live:
                        if cyc >= d and (cyc - d) % s == 0:
                            try:
                                next(g)
                            except StopIteration:
                                continue
                        nxt.append((g, s, d))
                    live = nxt
                    cyc += 1

            def pull(g, n):
                for _ in range(n):
                    try:
                        next(g)
                    except StopIteration:
                        return False
                return True

            g_attn = [attn_units(b) for b in range(B)]
            g_qkv = [qkv_units(b) for b in range(B)]
            g_out = [outproj_units(b) for b in range(B)]

            # prologue: batch 0 QKV chunk 0 first (its x DMA before wo/bo),
            # then pace attention(0) in at 1:3 while QKV(0) streams and
            # QKV(1) trickles in to fill attention(0)'s ACT-bound gaps
            pull(g_qkv[0], 1)
            late_const_dmas()
            pull(g_qkv[0], 16)
            drain((g_qkv[0], 1), (g_attn[0], 4), (g_qkv[1], 3, 52))
            for b in range(1, B):
                drain(
                    (g_qkv[b + 1] if b + 1 < B else None, 1),
                    (g_attn[b], 1),
                    (g_out[b - 1], 2, 10),
                    (g_out[b] if b == B - 1 else None, 2, 24),
                )
            drain((g_out[B - 1], 1))
    nc.finalize()
    return nc


def make_in_maps(x, attention_mask, Wq, bq, Wk, bk, Wv, bv, Wo, bo):
    x = np.asarray(x, dtype=np.float32)
    attention_mask = np.asarray(attention_mask, dtype=np.float32)
    Wq, Wk, Wv, Wo = (np.asarray(a, dtype=np.float32) for a in (Wq, Wk, Wv, Wo))
    bq, bk, bv, bo = (np.asarray(a, dtype=np.float32) for a in (bq, bk, bv, bo))

    xT = np.ascontiguousarray(x.transpose(0, 2, 1)).astype(BF)  # [B, D, S]
    # exp(mask[b,0,0,j]): [128 partitions, B*NJT, 1] column per (batch,
    # j-tile) for the va ones column, and [1, B, S] row layout for the
    # V em-scaling (folds the additive mask into the value/denom stream)
    m = attention_mask.reshape(B, S).reshape(B, NJT, 128)
    emcol = np.ascontiguousarray(
        np.exp(m.transpose(2, 0, 1).reshape(128, B * NJT, 1)))
    emrow = np.ascontiguousarray(
        np.exp(attention_mask.reshape(1, B, S)))

    def w3(W, cs):
        # [D, OF] -> [128, NKT, OF]: partition p holds rows {kt*128 + p}
        return np.ascontiguousarray(
            W[:, cs].reshape(NKT, 128, OF).transpose(1, 0, 2)).astype(BF)

    in_maps = []
    for c in range(NCORES):
        cs = slice(c * OF, (c + 1) * OF)
        in_maps.append({
            "xT": xT,
            "wq": w3(Wq, cs),
            "wk": w3(Wk, cs),
            "wv": w3(Wv, cs),
            "bq": np.ascontiguousarray(bq[cs]).reshape(OF, 1),
            "bk": np.ascontiguousarray(bk[cs]).reshape(OF, 1),
            "bv": np.ascontiguousarray(bv[cs]).reshape(OF, 1),
            "wo": np.ascontiguousarray(Wo[cs, :]).astype(BF),
            "emcol": emcol,
            "emrow": emrow,
        })
    return in_maps


def combine_outputs(results, bo):
    acc = np.zeros((B * S, D), dtype=np.float64)
    for r in results:
        acc += r["out"].astype(np.float64)
    acc += np.asarray(bo, dtype=np.float64)
    return acc.reshape(B, S, D).astype(np.float32)


_NC_CACHE = []


def _get_program():
    if not _NC_CACHE:
        _NC_CACHE.append(build_program())
    return _NC_CACHE[0]


def kernel(**inputs):
    nc = _get_program()
    in_maps = make_in_maps(**inputs)
    res = bass_utils.run_bass_kernel_spmd(
        nc, in_maps, core_ids=list(range(NCORES)))
    return combine_outputs(res.results, inputs["bo"])
